# revision 1
# baseline (speedup 1.0000x reference)
"""Pattention kernel for Trainium2 (8 NeuronCores, data-parallel over tokens).

Computes: out = (gelu_exact(X @ K^T) row-L2-normalized * sqrt(S)) @ V
  X: [4, 2048, 1024] f32, K: [4096, 1024] f32, V: [4096, 1024] f32
  out: [4, 2048, 1024] f32

Strategy:
  - Flatten tokens: T_total = 8192; shard 1024 tokens per core (no collectives:
    the L2 norm is over S, which stays whole on every core).
  - Host-side prep: transpose X-shard and K to put the contraction dim (D) on
    partitions, cast matmul operands to fp16 (PE runs fp16 at full rate; 2^-11
    rounding keeps rel-err ~1e-3).
  - On-chip per core, for each block of 256 tokens:
      scores^T chunk [128s, 256t] = sum_d KT[d,s-chunk].T @ XT[d, t-block]  (PE)
      g = gelu(scores^T)  (ACT, exact-erf gelu, psum -> sbuf fp16)
      g2 = g*g            (DVE)
      sumsq[1, 256] += ones.T @ g2   (PE, accumulated over all 32 s-chunks)
      U[t-tile, dv] = sum_s g^T.T @ V (PE)   [normalization deferred]
      out = U * (sqrt(S) * rsqrt(sumsq))     (ACT copy with per-partition scale)
"""

import numpy as np

B, L, D = 4, 2048, 1024
S = 4096
DV = 1024
NCORES = 8
T = (B * L) // NCORES          # 1024 tokens per core
TB = 256                       # token block
N_TB = T // TB                 # 4
N_D = D // 128                 # 8 contraction chunks
N_S = S // 128                 # 32 s-chunks
N_TT = TB // 128               # 2 token tiles per block
N_DVB = DV // 512              # 2 output column blocks

_cache = {}


def _build():
    import concourse.mybir as mybir
    import concourse.tile as tile
    from concourse import bacc

    f16 = mybir.dt.float16
    f32 = mybir.dt.float32
    AF = mybir.ActivationFunctionType

    nc = bacc.Bacc("TRN2", target_bir_lowering=False, debug=False,
                   num_devices=NCORES)
    kt_d = nc.dram_tensor("kt", [N_D, 128, S], f16, kind="ExternalInput").ap()
    xt_d = nc.dram_tensor("xt", [N_D, 128, T], f16, kind="ExternalInput").ap()
    v_d = nc.dram_tensor("v", [N_S, 128, DV], f16, kind="ExternalInput").ap()
    o_d = nc.dram_tensor("o", [T, DV], f32, kind="ExternalOutput").ap()

    with tile.TileContext(nc) as tc:
        with tc.tile_pool(name="consts", bufs=1) as consts, \
             tc.tile_pool(name="gpool", bufs=2) as gpool, \
             tc.tile_pool(name="g2pool", bufs=4) as g2pool, \
             tc.tile_pool(name="opool", bufs=3) as opool, \
             tc.tile_pool(name="smpool", bufs=2) as smpool, \
             tc.tile_pool(name="ps_sc", bufs=3, space="PSUM") as ps_sc, \
             tc.tile_pool(name="ps_ss", bufs=2, space="PSUM") as ps_ss, \
             tc.tile_pool(name="ps_u", bufs=3, space="PSUM") as ps_u:

            # resident operands; kt split into column blocks so the first
            # s-chunks can start after ~4MB of DMA
            kt = [[consts.tile([128, 1024], f16, tag=f"kt{d}_{q}", name=f"kt{d}_{q}")
                   for q in range(4)] for d in range(N_D)]
            xt = [consts.tile([128, T], f16, tag=f"xt{d}", name=f"xt{d}")
                  for d in range(N_D)]
            v = [consts.tile([128, DV], f16, tag=f"v{s}", name=f"v{s}")
                 for s in range(N_S)]
            for d in range(N_D):
                nc.sync.dma_start(out=kt[d][0], in_=kt_d[d, :, 0:1024])
            for d in range(N_D):
                nc.sync.dma_start(out=xt[d], in_=xt_d[d])
            for q in range(1, 4):
                for d in range(N_D):
                    nc.sync.dma_start(out=kt[d][q], in_=kt_d[d, :, q * 1024:(q + 1) * 1024])
            for s in range(N_S):
                nc.sync.dma_start(out=v[s], in_=v_d[s])
            ones = consts.tile([128, 1], f16, name="ones")
            nc.vector.memset(ones, 1.0)

            for tb in range(N_TB):
                t0 = tb * TB
                ss_ps = ps_ss.tile([1, TB], f32, tag="ss", name=f"ss{tb}")
                gts = []
                for sc in range(N_S):
                    q, r = divmod(sc * 128, 1024)
                    sc_ps = ps_sc.tile([128, TB], f32, tag="sc", name=f"sc{tb}_{sc}")
                    for d in range(N_D):
                        nc.tensor.matmul(sc_ps, lhsT=kt[d][q][:, r:r + 128],
                                         rhs=xt[d][:, t0:t0 + TB],
                                         start=(d == 0), stop=(d == N_D - 1))
                    g = gpool.tile([128, TB], f16, tag=f"g{sc}", name=f"g{tb}_{sc}")
                    nc.scalar.activation(out=g, in_=sc_ps, func=AF.Gelu)
                    g2 = g2pool.tile([128, TB], f16, tag="g2", name=f"g2{tb}_{sc}")
                    nc.vector.tensor_mul(g2, g, g)
                    nc.tensor.matmul(ss_ps, lhsT=ones, rhs=g2,
                                     start=(sc == 0), stop=(sc == N_S - 1),
                                     skip_group_check=True)
                    gts.append(g)

                # sumsq -> per-partition scale: scal = sqrt(S) / sqrt(sumsq)
                # (one Newton step on the reciprocal-sqrt for accuracy)
                ss_sb = smpool.tile([1, TB], f32, tag="sssb", name=f"sssb{tb}")
                nc.vector.tensor_copy(out=ss_sb, in_=ss_ps)
                sm = smpool.tile([128, 16], f32, tag="sm", name=f"sm{tb}")
                for tt in range(N_TT):
                    nc.sync.dma_start(out=sm[:, tt:tt + 1],
                                      in_=ss_sb[0:1, tt * 128:(tt + 1) * 128])
                nc.scalar.activation(out=sm[:, 2:4], in_=sm[:, 0:2], func=AF.Sqrt)
                nc.vector.reciprocal(out=sm[:, 4:6], in_=sm[:, 2:4])
                nc.vector.tensor_mul(sm[:, 6:8], sm[:, 4:6], sm[:, 4:6])
                nc.vector.tensor_mul(sm[:, 8:10], sm[:, 0:2], sm[:, 6:8])
                nc.vector.tensor_scalar(out=sm[:, 10:12], in0=sm[:, 8:10],
                                        scalar1=-0.5, scalar2=1.5,
                                        op0=mybir.AluOpType.mult,
                                        op1=mybir.AluOpType.add)
                nc.vector.tensor_mul(sm[:, 12:14], sm[:, 4:6], sm[:, 10:12])
                nc.vector.tensor_scalar(out=sm[:, 12:14], in0=sm[:, 12:14],
                                        scalar1=float(np.sqrt(S)), scalar2=None,
                                        op0=mybir.AluOpType.mult)

                for tt in range(N_TT):
                    for dvb in range(N_DVB):
                        u_ps = ps_u.tile([128, 512], f32, tag="u",
                                         name=f"u{tb}_{tt}_{dvb}")
                        for sc in range(N_S):
                            nc.tensor.matmul(u_ps,
                                             lhsT=gts[sc][:, tt * 128:(tt + 1) * 128],
                                             rhs=v[sc][:, dvb * 512:(dvb + 1) * 512],
                                             start=(sc == 0), stop=(sc == N_S - 1),
                                             skip_group_check=True)
                        o_sb = opool.tile([128, 512], f32, tag="o",
                                          name=f"o{tb}_{tt}_{dvb}")
                        nc.scalar.activation(out=o_sb, in_=u_ps, func=AF.Copy,
                                             scale=sm[:, 12 + tt:13 + tt])
                        nc.sync.dma_start(
                            out=o_d[t0 + tt * 128:t0 + (tt + 1) * 128,
                                    dvb * 512:(dvb + 1) * 512],
                            in_=o_sb)

    nc.compile()
    return nc


def _prep_in_maps(inputs, key_param_tokens, value_param_tokens):
    x = np.ascontiguousarray(np.asarray(inputs, dtype=np.float32).reshape(B * L, D))
    k = np.asarray(key_param_tokens, dtype=np.float32)
    vv = np.asarray(value_param_tokens, dtype=np.float32)

    kt_h = np.ascontiguousarray(k.T).astype(np.float16).reshape(N_D, 128, S)
    v_h = vv.astype(np.float16).reshape(N_S, 128, DV)
    in_maps = []
    for c in range(NCORES):
        xs = x[c * T:(c + 1) * T]                       # [T, D]
        xt_h = np.ascontiguousarray(xs.T).astype(np.float16).reshape(N_D, 128, T)
        in_maps.append({"kt": kt_h, "xt": xt_h, "v": v_h})
    return in_maps


def kernel(inputs, key_param_tokens, value_param_tokens):
    from concourse.bass_utils import run_bass_kernel_spmd

    if "nc" not in _cache:
        _cache["nc"] = _build()
    nc = _cache["nc"]

    in_maps = _prep_in_maps(inputs, key_param_tokens, value_param_tokens)
    res = run_bass_kernel_spmd(nc, in_maps, core_ids=list(range(NCORES)))
    out = np.concatenate([res.results[c]["o"] for c in range(NCORES)], axis=0)
    return out.reshape(B, L, DV)


# revision 3
# speedup vs baseline: 24.2750x; 24.2750x over previous
"""Pattention kernel for Trainium2 (8 NeuronCores, data-parallel over tokens).

Computes: out = (gelu_exact(X @ K^T) row-L2-normalized * sqrt(S)) @ V
  X: [4, 2048, 1024] f32, K: [4096, 1024] f32, V: [4096, 1024] f32
  out: [4, 2048, 1024] f32

Strategy:
  - Flatten tokens: T_total = 8192; shard 1024 tokens per core (no collectives:
    the L2 norm is over S, which stays whole on every core).
  - Host-side prep: transpose X-shard and K to put the contraction dim (D) on
    partitions, cast matmul operands to fp16 (PE runs fp16 at full rate; 2^-11
    rounding keeps rel-err ~1e-3).
  - On-chip per core, for each block of 256 tokens:
      scores^T chunk [128s, 256t] = sum_d KT[d,s-chunk].T @ XT[d, t-block]  (PE)
      g = gelu(scores^T)  (ACT, exact-erf gelu, psum -> sbuf fp16)
      g2 = g*g            (DVE)
      sumsq[1, 256] += ones.T @ g2   (PE, accumulated over all 32 s-chunks)
      U[t-tile, dv] = sum_s g^T.T @ V (PE)   [normalization deferred]
      out = U * (sqrt(S) * rsqrt(sumsq))     (ACT copy with per-partition scale)
"""

import numpy as np

B, L, D = 4, 2048, 1024
S = 4096
DV = 1024
NCORES = 8
T = (B * L) // NCORES          # 1024 tokens per core
TB = 256                       # token block
N_TB = T // TB                 # 4
N_D = D // 128                 # 8 contraction chunks
N_S = S // 128                 # 32 s-chunks
N_TT = TB // 128               # 2 token tiles per block
N_DVB = DV // 512              # 2 output column blocks

_cache = {}


def _build(reps=1):
    import concourse.mybir as mybir
    import concourse.tile as tile
    from concourse import bacc

    f16 = mybir.dt.float16
    f32 = mybir.dt.float32
    AF = mybir.ActivationFunctionType

    nc = bacc.Bacc("TRN2", target_bir_lowering=False, debug=False,
                   num_devices=NCORES)
    kt_d = nc.dram_tensor("kt", [N_D, 128, S], f16, kind="ExternalInput").ap()
    xt_d = nc.dram_tensor("xt", [N_D, 128, T], f16, kind="ExternalInput").ap()
    v_d = nc.dram_tensor("v", [N_S, 128, DV], f16, kind="ExternalInput").ap()
    o_d = nc.dram_tensor("o", [T, DV], f32, kind="ExternalOutput").ap()

    with tile.TileContext(nc) as tc:
        with tc.tile_pool(name="consts", bufs=1) as consts, \
             tc.tile_pool(name="gpool", bufs=2) as gpool, \
             tc.tile_pool(name="g2pool", bufs=4) as g2pool, \
             tc.tile_pool(name="opool", bufs=3) as opool, \
             tc.tile_pool(name="smpool", bufs=2) as smpool, \
             tc.tile_pool(name="ps_sc", bufs=3, space="PSUM") as ps_sc, \
             tc.tile_pool(name="ps_ss", bufs=2, space="PSUM") as ps_ss, \
             tc.tile_pool(name="ps_u", bufs=3, space="PSUM") as ps_u:

            # resident operands; kt split into column blocks so the first
            # s-chunks can start after ~4MB of DMA
            kt = [[consts.tile([128, 1024], f16, tag=f"kt{d}_{q}", name=f"kt{d}_{q}")
                   for q in range(4)] for d in range(N_D)]
            xt = [consts.tile([128, T], f16, tag=f"xt{d}", name=f"xt{d}")
                  for d in range(N_D)]
            v = [consts.tile([128, DV], f16, tag=f"v{s}", name=f"v{s}")
                 for s in range(N_S)]
            for d in range(N_D):
                nc.sync.dma_start(out=kt[d][0], in_=kt_d[d, :, 0:1024])
            for d in range(N_D):
                nc.sync.dma_start(out=xt[d], in_=xt_d[d])
            for q in range(1, 4):
                for d in range(N_D):
                    nc.sync.dma_start(out=kt[d][q], in_=kt_d[d, :, q * 1024:(q + 1) * 1024])
            for s in range(N_S):
                nc.sync.dma_start(out=v[s], in_=v_d[s])
            ones = consts.tile([128, 1], f16, name="ones")
            nc.vector.memset(ones, 1.0)

            for rep_tb in range(reps * N_TB):
                tb = rep_tb % N_TB
                t0 = tb * TB
                ss_ps = ps_ss.tile([1, TB], f32, tag="ss", name=f"ss{rep_tb}")
                gts = []
                for sc in range(N_S):
                    q, r = divmod(sc * 128, 1024)
                    sc_ps = ps_sc.tile([128, TB], f32, tag="sc", name=f"sc{tb}_{sc}")
                    for d in range(N_D):
                        nc.tensor.matmul(sc_ps, lhsT=kt[d][q][:, r:r + 128],
                                         rhs=xt[d][:, t0:t0 + TB],
                                         start=(d == 0), stop=(d == N_D - 1))
                    g = gpool.tile([128, TB], f16, tag=f"g{sc}", name=f"g{tb}_{sc}")
                    nc.scalar.activation(out=g, in_=sc_ps, func=AF.Gelu)
                    g2 = g2pool.tile([128, TB], f16, tag="g2", name=f"g2{tb}_{sc}")
                    nc.vector.tensor_mul(g2, g, g)
                    nc.tensor.matmul(ss_ps, lhsT=ones, rhs=g2,
                                     start=(sc == 0), stop=(sc == N_S - 1),
                                     skip_group_check=True)
                    gts.append(g)

                # sumsq -> per-partition scale: scal = sqrt(S) / sqrt(sumsq)
                # (one Newton step on the reciprocal-sqrt for accuracy)
                ss_sb = smpool.tile([1, TB], f32, tag="sssb", name=f"sssb{tb}")
                nc.vector.tensor_copy(out=ss_sb, in_=ss_ps)
                sm = smpool.tile([128, 16], f32, tag="sm", name=f"sm{tb}")
                for tt in range(N_TT):
                    nc.sync.dma_start(out=sm[:, tt:tt + 1],
                                      in_=ss_sb[0:1, tt * 128:(tt + 1) * 128])
                nc.scalar.activation(out=sm[:, 2:4], in_=sm[:, 0:2], func=AF.Sqrt)
                nc.vector.reciprocal(out=sm[:, 4:6], in_=sm[:, 2:4])
                nc.vector.tensor_mul(sm[:, 6:8], sm[:, 4:6], sm[:, 4:6])
                nc.vector.tensor_mul(sm[:, 8:10], sm[:, 0:2], sm[:, 6:8])
                nc.vector.tensor_scalar(out=sm[:, 10:12], in0=sm[:, 8:10],
                                        scalar1=-0.5, scalar2=1.5,
                                        op0=mybir.AluOpType.mult,
                                        op1=mybir.AluOpType.add)
                nc.vector.tensor_mul(sm[:, 12:14], sm[:, 4:6], sm[:, 10:12])
                nc.vector.tensor_scalar(out=sm[:, 12:14], in0=sm[:, 12:14],
                                        scalar1=float(np.sqrt(S)), scalar2=None,
                                        op0=mybir.AluOpType.mult)

                for tt in range(N_TT):
                    for dvb in range(N_DVB):
                        u_ps = ps_u.tile([128, 512], f32, tag="u",
                                         name=f"u{tb}_{tt}_{dvb}")
                        for sc in range(N_S):
                            nc.tensor.matmul(u_ps,
                                             lhsT=gts[sc][:, tt * 128:(tt + 1) * 128],
                                             rhs=v[sc][:, dvb * 512:(dvb + 1) * 512],
                                             start=(sc == 0), stop=(sc == N_S - 1),
                                             skip_group_check=True)
                        o_sb = opool.tile([128, 512], f32, tag="o",
                                          name=f"o{tb}_{tt}_{dvb}")
                        nc.scalar.activation(out=o_sb, in_=u_ps, func=AF.Copy,
                                             scale=sm[:, 12 + tt:13 + tt])
                        nc.sync.dma_start(
                            out=o_d[t0 + tt * 128:t0 + (tt + 1) * 128,
                                    dvb * 512:(dvb + 1) * 512],
                            in_=o_sb)

    nc.compile()
    return nc


def _prep_in_maps(inputs, key_param_tokens, value_param_tokens):
    x = np.ascontiguousarray(np.asarray(inputs, dtype=np.float32).reshape(B * L, D))
    k = np.asarray(key_param_tokens, dtype=np.float32)
    vv = np.asarray(value_param_tokens, dtype=np.float32)

    kt_h = np.ascontiguousarray(k.T).astype(np.float16).reshape(N_D, 128, S)
    v_h = vv.astype(np.float16).reshape(N_S, 128, DV)
    in_maps = []
    for c in range(NCORES):
        xs = x[c * T:(c + 1) * T]                       # [T, D]
        xt_h = np.ascontiguousarray(xs.T).astype(np.float16).reshape(N_D, 128, T)
        in_maps.append({"kt": kt_h, "xt": xt_h, "v": v_h})
    return in_maps


def kernel(inputs, key_param_tokens, value_param_tokens):
    from concourse.bass_utils import run_bass_kernel_spmd

    if "nc" not in _cache:
        _cache["nc"] = _build()
    nc = _cache["nc"]

    in_maps = _prep_in_maps(inputs, key_param_tokens, value_param_tokens)
    res = run_bass_kernel_spmd(nc, in_maps, core_ids=list(range(NCORES)))
    out = np.concatenate([res.results[c]["o"] for c in range(NCORES)], axis=0)
    return out.reshape(B, L, DV)
